# revision 3
# baseline (speedup 1.0000x reference)
"""FM pairwise-interaction layer on 8 Trainium2 NeuronCores.

out[b, p] = x[b, I1[p]] * x[b, I2[p]] * dot(w[I1[p]], w[I2[p]])   for all
P = 512*511/2 = 130816 strict upper-triangle pairs, batch 1024.

Strategy (data-parallel over batch, 128 rows per core):
  *  p-space is ordered by j1-blocks: block j1 covers columns
     [off(j1), off(j1)+n), n = 511-j1, with j2 = j1+1..511 contiguous.
  *  Host precomputes WP[k, p] = w[I1[p], k] * w[I2[p], k]  (weight-derived
     only, [4, P] fp32) and splits it into bf16 hi/lo.  Host also ships
     x.T in bf16 hi/lo, replicated per k, as the stationary operand.
  *  Per block, ONE K=12 bf16 matmul computes
        psum[b, c] = sum_k x[b, j1] * WP[k, off+c]  =  x[b, j1] * wdot[p]
     exactly-ish (hi*hi + hi*lo + lo*hi pairs, fp32 PSUM accumulate,
     ~1e-5 rel err).  Operands sit at 32-aligned partition groups.
  *  DVE multiplies psum by the plain slice x[:, j1+1:512] (tensor_mul;
     adjacent blocks are merged into one 2D-AP op), writing flat-p staging
     chunks that are DMAd to DRAM as ~4 MB transfers (memory roofline).
"""

import numpy as np
import ml_dtypes

import concourse.bass as bass
import concourse.mybir as mybir
from concourse import bacc
from concourse.tile import TileContext
import concourse.bass_utils as bass_utils

NF = 512          # features
K = 4             # latent dim
B = 1024          # batch
NCORES = 8
BS = B // NCORES  # 128 batch rows per core
P = NF * (NF - 1) // 2  # 130816 pairs
CH = 4096         # staging chunk columns (fp32) -> 4 MB per DMA
PAIR_MERGE = True
TRACE = False
LAST_RESULT = {}
_last_in_maps = None

_bf16 = ml_dtypes.bfloat16


def _off(j1):
    return j1 * (NF - 1) - j1 * (j1 - 1) // 2


_GOFF = [_off(0), _off(128), _off(256), _off(384), P]
_GW = [_GOFF[g + 1] - _GOFF[g] for g in range(4)]  # 57280, 40896, 24512, 8128
_WPAD = 8


def _split_bf16(a):
    hi = a.astype(_bf16)
    lo = (a - hi.astype(np.float32)).astype(_bf16)
    return hi, lo


def _build_nc():
    nc = bacc.Bacc("TRN2", target_bir_lowering=False, debug=False,
                   num_devices=NCORES)
    f32 = mybir.dt.float32
    bf16 = mybir.dt.bfloat16

    x_d = nc.dram_tensor("x", (BS, NF), f32, kind="ExternalInput").ap()
    xt_d = nc.dram_tensor("xt12", (4, 12, 128 * 128), bf16,
                          kind="ExternalInput").ap()
    wp_d = [nc.dram_tensor(f"wp{g}", (12, _GW[g] + _WPAD), bf16,
                           kind="ExternalInput").ap() for g in range(4)]
    out_d = nc.dram_tensor("out", (BS, P), f32, kind="ExternalOutput").ap()

    def ap2d(sliced, dims):
        c = sliced.copy()
        v = c.ap
        part = [list(v[0])]
        while len(v) > 0:
            v.pop()
        for d in part + [list(x) for x in dims]:
            v.append(d)
        c.ap = v
        return c

    with TileContext(nc) as tc:
        with tc.tile_pool(name="sb", bufs=1) as sb, \
             tc.tile_pool(name="stg", bufs=2) as stg, \
             tc.tile_pool(name="ps", bufs=4, space="PSUM") as ps:

            xs = sb.tile([128, NF + 8], f32, tag="xs")
            nc.vector.memset(xs[:, NF:NF + 8], 0.0)
            nc.sync.dma_start(out=xs[:, 0:NF], in_=x_d[:])

            xt = sb.tile([128, 128 * 128], bf16, tag="xt")
            wp = sb.tile([128, _GW[0] + _WPAD], bf16, tag="wp")
            for g in range(4):
                nc.sync.dma_start(out=xt[32 * g:32 * g + 12, :], in_=xt_d[g])
                nc.sync.dma_start(out=wp[32 * g:32 * g + 12, 0:_GW[g] + _WPAD],
                                  in_=wp_d[g][:])

            chunk_start = 0
            chunk_end = min(CH, P)
            stage = stg.tile([128, CH], f32, tag="stage")

            def flush():
                nonlocal chunk_start, chunk_end, stage
                nc.sync.dma_start(out=out_d[:, chunk_start:chunk_end],
                                  in_=stage[:, 0:chunk_end - chunk_start])
                chunk_start = chunk_end
                chunk_end = min(chunk_start + CH, P)
                if chunk_start < P:
                    stage = stg.tile([128, CH], f32, tag="stage")

            def lhs(j1):
                g = j1 // 128
                r = j1 - 128 * g
                return xt[32 * g:32 * g + 12, r * 128:(r + 1) * 128]

            def rhs(j1, n):
                g = j1 // 128
                lo = _off(j1) - _GOFF[g]
                return wp[32 * g:32 * g + 12, lo:lo + n]

            j1 = 0
            while j1 < NF - 1:
                n = NF - 1 - j1
                o = _off(j1)
                g = j1 // 128
                pair_ok = (PAIR_MERGE and j1 + 1 < NF - 1
                           and (j1 + 1) // 128 == g
                           and n <= 512
                           and _off(j1 + 2) + 1 <= chunk_end)
                if pair_ok:
                    psum = ps.tile([128, 1024], f32, tag="psum")
                    nc.tensor.matmul(psum[:, 0:n], lhs(j1), rhs(j1, n),
                                     start=True, stop=True,
                                     tile_position=(32 * g, 0))
                    # second block: n cols too (1 past its end; WP zero-padded)
                    nc.tensor.matmul(psum[:, 512:512 + n], lhs(j1 + 1),
                                     rhs(j1 + 1, n),
                                     start=True, stop=True,
                                     tile_position=(32 * g, 0))
                    lo = o - chunk_start
                    out_ap = ap2d(stage[:, lo:lo + 1], [[n, 2], [1, n]])
                    in0_ap = ap2d(psum[:, 0:1], [[512, 2], [1, n]])
                    in1_ap = ap2d(xs[:, j1 + 1:j1 + 2], [[1, 2], [1, n]])
                    nc.vector.tensor_mul(out=out_ap, in0=in0_ap, in1=in1_ap)
                    j1 += 2
                    if _off(j1) >= chunk_end:
                        flush()
                else:
                    psum = ps.tile([128, 1024], f32, tag="psum")
                    nc.tensor.matmul(psum[:, 0:n], lhs(j1), rhs(j1, n),
                                     start=True, stop=True,
                                     tile_position=(32 * g, 0))
                    pos = o
                    while pos < o + n:
                        take = min(o + n, chunk_end) - pos
                        nc.vector.tensor_mul(
                            out=stage[:, pos - chunk_start:pos - chunk_start + take],
                            in0=psum[:, pos - o:pos - o + take],
                            in1=xs[:, j1 + 1 + pos - o:j1 + 1 + pos - o + take])
                        pos += take
                        if pos == chunk_end:
                            flush()
                    j1 += 1
            if chunk_start < P:
                flush()

    nc.compile()
    return nc


_NC_CACHE = None


def kernel(x, weight):
    global _NC_CACHE, LAST_RESULT
    x = np.ascontiguousarray(x, dtype=np.float32)
    weight = np.ascontiguousarray(weight, dtype=np.float32)
    assert x.shape == (B, NF) and weight.shape == (NF, K)

    # ---- host-side weight-derived constants
    i1, i2 = np.triu_indices(NF, k=1)
    wp_full = (weight[i1] * weight[i2]).T.astype(np.float32)  # [K, P]
    wph, wpl = _split_bf16(wp_full)
    wp_in = {}
    for g in range(4):
        arr = np.zeros((12, _GW[g] + _WPAD), dtype=_bf16)
        sl = slice(_GOFF[g], _GOFF[g + 1])
        for k in range(K):
            arr[3 * k + 0, 0:_GW[g]] = wph[k, sl]
            arr[3 * k + 1, 0:_GW[g]] = wpl[k, sl]
            arr[3 * k + 2, 0:_GW[g]] = wph[k, sl]
        wp_in[f"wp{g}"] = arr

    # ---- per-core inputs
    in_maps = []
    for c in range(NCORES):
        xc = x[c * BS:(c + 1) * BS]           # [128, 512]
        xct = np.ascontiguousarray(xc.T)      # [512, 128]
        xh, xl = _split_bf16(xct)
        xt12 = np.empty((4, 12, 128 * 128), dtype=_bf16)
        for g in range(4):
            fh = xh[128 * g:128 * (g + 1)].reshape(-1)
            fl = xl[128 * g:128 * (g + 1)].reshape(-1)
            for k in range(K):
                xt12[g, 3 * k + 0] = fh
                xt12[g, 3 * k + 1] = fh
                xt12[g, 3 * k + 2] = fl
        m = {"x": xc, "xt12": xt12}
        m.update(wp_in)
        in_maps.append(m)

    global _last_in_maps
    _last_in_maps = in_maps
    if _NC_CACHE is None:
        _NC_CACHE = _build_nc()
    nc = _NC_CACHE

    res = bass_utils.run_bass_kernel_spmd(nc, in_maps,
                                          core_ids=list(range(NCORES)),
                                          trace=TRACE)
    LAST_RESULT = {"exec_time_ns": res.exec_time_ns,
                   "trace": res.instructions_and_trace}
    out = np.concatenate([r["out"] for r in res.results], axis=0)
    return out
